# revision 13
# baseline (speedup 1.0000x reference)
# Trainium2 Bass kernel for a 2-layer GraphSAGE encoder (SAGEConv mean aggr).
#
#   h   = relu(mean_nbr(x) @ W1_l + b1 + x @ W1_r)
#   out = mean_nbr(h) @ W2_l + b2 + h @ W2_r
#
# Sharding: data-parallel over destination nodes (8 cores); host permutes node
# ids (degree-balanced snake deal), pads N to 8*shard, core k owns dst rows
# [k*shard,(k+1)*shard).
#
# Layer 1 messages x[src] are host-packed into a contiguous bf16 slot stream
# and streamed sequentially (no runtime gather); one PE matmul per 128-slot
# tile against segment matrix S1 (holding 1/deg) accumulates agg per dst.
#
# Layer 2 messages (h[src]) are gathered at runtime. To overlap the gather
# with layer 1, source nodes are split into K chunks by local index range;
# tiles are chunk-pure, and h is AllGathered chunk-by-chunk as soon as each
# chunk's dense layer completes, so chunk-c gathers run while the layer-1
# stream for later chunks is still going.  Gathered rows are full h rows
# (256B singles; per-chunk row index fits int16), aggregated per (chunk,
# batch) bucket into PSUM and accumulated into an SBUF f32 accumulator.
# The layer-2 matmuls for chunk c are emitted one chunk-span later so the PE
# stream never stalls on gather data and m2 pool buffers recycle in time.
# Small linear weights are replicated; outputs written [OC, shard]
# column-major and transposed on the host.
import os
import sys
import numpy as np

for _p in ("/opt/trn_rl_repo",):
    if _p not in sys.path and os.path.isdir(_p):
        sys.path.append(_p)

import concourse.bass as bass
import concourse.bacc as bacc
import concourse.mybir as mybir
from concourse import tile
from concourse.bass_utils import run_bass_kernel_spmd

F32 = mybir.dt.float32
BF16 = mybir.dt.bfloat16
I16 = mybir.dt.int16
NP_BF16 = mybir.dt.np(BF16)

N_CORES = 8
BATCH = 128      # dst nodes per aggregation batch (PSUM tile width)
K = 2            # source chunks (AllGather pipeline depth)
CT = 8           # L2 gather call size in 128-slot tiles
CT1 = 32         # L1 stream chunk size in tiles (1MB DMAs)
SPARTS = 8       # S1 load splits (batch-aligned)


def _cdiv(a, b):
    return -(-a // b)


# ----------------------------------------------------------------------------
# Host-side graph preprocessing (index manipulation / data staging only).
# ----------------------------------------------------------------------------
def _preprocess(x, edge_index):
    x = np.asarray(x, np.float32)
    ei = np.asarray(edge_index, np.int64)
    N, C = x.shape
    E = ei.shape[1]
    src, dst = ei[0], ei[1]

    shard = _cdiv(_cdiv(N, N_CORES), BATCH) * BATCH
    NP = shard * N_CORES
    NBT = shard // BATCH

    # chunk boundaries in batch units (Ke chunks over NBT batches)
    Ke = max(1, min(K, NBT))
    cb = [round(c * NBT / Ke) for c in range(Ke + 1)]
    chunk_nb = [cb[c + 1] - cb[c] for c in range(Ke)]

    deg = np.bincount(dst, minlength=N).astype(np.int64)
    recip_full = (1.0 / np.maximum(deg, 1)).astype(np.float32)

    # Degree-balanced snake deal over (core, batch-of-128) bins.
    nbins = N_CORES * NBT
    order = np.argsort(-deg, kind="stable")
    i = np.arange(N)
    r = i // nbins
    p = i % nbins
    binidx = np.where(r % 2 == 0, p, nbins - 1 - p)
    core_b = binidx % N_CORES
    bat_b = binidx // N_CORES
    newid = core_b * shard + bat_b * BATCH + r
    perm = np.empty(N, np.int64)
    perm[order] = newid

    psrc = perm[src]
    pdst = perm[dst]

    x_tab = np.zeros((NP, C), np.float32)
    x_tab[perm] = x
    x_tab16 = x_tab.astype(NP_BF16)
    recip_bc = np.zeros(NP, np.float32)
    recip_bc[perm] = recip_full

    # source chunk of each edge (by src's local batch index on its owner core)
    src_local = psrc % shard
    src_bat = src_local // BATCH
    cb_arr = np.asarray(cb[1:])                        # upper bounds
    src_chunk = np.searchsorted(cb_arr, src_bat, side="right")
    # row of src within h_chunk_c: owner_core * chunk_nodes + (local - start)
    src_core = psrc // shard
    src_row = np.empty(E, np.int64)
    for c in range(Ke):
        m = src_chunk == c
        src_row[m] = (src_core[m] * (chunk_nb[c] * BATCH)
                      + src_local[m] - cb[c] * BATCH)
        assert src_row[m].max(initial=0) < 32768, "chunk row exceeds int16"

    core_of = pdst // shard
    local = pdst % shard

    # per-(core, local-dst, chunk) degree
    keyd = (core_of * shard + local) * Ke + src_chunk
    degs = np.bincount(keyd, minlength=N_CORES * shard * Ke)
    degs = degs.reshape(N_CORES, shard, Ke)              # [core, local, chunk]
    assert degs.sum(axis=2).max() <= 128, "single dst degree exceeds one tile"

    # Structural tile plan, uniform across cores. Stream (batch-major) order:
    # for each batch b, for each chunk c, greedily pack the 128 dst columns
    # (zero-width allowed; every dst gets a column in every bucket) into tiles
    # where every core's slot total fits 128.
    tiles = []            # (b, c, a, w) in stream-emission order
    bucket_tiles = {}     # (b, c) -> list of tile ids
    for b in range(NBT):
        for c in range(Ke):
            d = degs[:, b * BATCH:(b + 1) * BATCH, c]   # [core, 128]
            csum = np.concatenate(
                [np.zeros((N_CORES, 1), np.int64), np.cumsum(d, axis=1)],
                axis=1)
            tl = []
            a = 0
            while a < BATCH:
                base = csum[:, a]
                w = 1
                while a + w < BATCH and \
                        ((csum[:, a + w + 1] - base) <= 128).all():
                    w += 1
                tl.append((b, c, a, w))
                a += w
            bucket_tiles[(b, c)] = list(range(len(tiles), len(tiles) + len(tl)))
            tiles.extend(tl)
    T = len(tiles)

    scol_off = np.zeros(T + 1, np.int64)
    for t, (b, c, a, w) in enumerate(tiles):
        scol_off[t + 1] = scol_off[t] + w
    SCOLS = int(scol_off[-1])
    assert SCOLS == NBT * Ke * BATCH

    # gather (chunk-major) tile order
    gorder = []           # gather position -> tile id
    for c in range(Ke):
        for b in range(NBT):
            gorder.extend(bucket_tiles[(b, c)])
    gpos = np.empty(T, np.int64)
    gpos[np.asarray(gorder)] = np.arange(T)
    chunk_t0 = [0] * (Ke + 1)    # gather-pos range per chunk
    for c in range(Ke):
        nt_c = sum(len(bucket_tiles[(b, c)]) for b in range(NBT))
        chunk_t0[c + 1] = chunk_t0[c] + nt_c

    # --- per-core slot/S content -------------------------------------------
    # edges sorted by (core, local dst, chunk) with rank within each group
    keye = (core_of * shard + local) * Ke + src_chunk
    ordr = np.argsort(keye, kind="stable")
    psrc_s = psrc[ordr]
    srcrow_s = src_row[ordr]
    keye_s = keye[ordr]
    starts = np.concatenate([[0], np.cumsum(degs.reshape(-1))])
    rank = np.arange(E) - starts[keye_s]
    core_e = keye_s // (shard * Ke)
    loc_e = (keye_s // Ke) % shard
    chk_e = keye_s % Ke

    def wrap_idx(a_):
        return np.ascontiguousarray(
            np.tile(a_.reshape(-1, 16).T, (8, 1)).astype(np.int16))

    per_core = []
    for k in range(N_CORES):
        slot_base = np.zeros((shard, Ke), np.int64)
        S1 = np.zeros((128, SCOLS), np.float32)
        for t, (b, c, a, w) in enumerate(tiles):
            dloc = b * BATCH + a
            dsl = degs[k, dloc:dloc + w, c]
            offs = np.concatenate([[0], np.cumsum(dsl)])
            assert offs[-1] <= 128
            slot_base[dloc:dloc + w, c] = t * 128 + offs[:-1]
            nz = np.nonzero(dsl)[0]
            for j in nz:
                S1[offs[j]:offs[j + 1], scol_off[t] + j] = \
                    recip_bc[k * shard + dloc + j]
        m = core_e == k
        slot = slot_base[loc_e[m], chk_e[m]] + rank[m]
        slotsrc = np.zeros(T * 128, np.int64)
        slotsrc[slot] = psrc_s[m]
        slotrow = np.zeros(T * 128, np.int64)
        slotrow[slot] = srcrow_s[m]

        src_grid = slotsrc.reshape(T, 128).T            # [128, T] stream order
        msgs1 = x_tab16[src_grid].reshape(128, T * C)   # [128, T*C] bf16

        # gather-order idx: per gather position g, the tile gorder[g]
        rows_g = slotrow.reshape(T, 128)[np.asarray(gorder)]  # [T,128] g-order
        idx2 = wrap_idx(rows_g.reshape(-1))             # [128, T*8] int16

        ent = {
            "msgs1": np.ascontiguousarray(msgs1),
            "idx2": idx2,
            "S1": S1.astype(NP_BF16),
            "xT_sh": np.ascontiguousarray(
                x_tab16.T[:, k * shard:(k + 1) * shard]),
        }
        per_core.append(ent)

    meta = dict(NP=NP, shard=shard, NBT=NBT, C=C, T=T, SCOLS=SCOLS, K=Ke,
                tiles=tiles, bucket_tiles=bucket_tiles,
                scol_off=scol_off.tolist(), cb=cb, chunk_nb=chunk_nb,
                gorder=gorder, gpos=gpos.tolist(), chunk_t0=chunk_t0)
    return per_core, perm, meta


# ----------------------------------------------------------------------------
# Bass program builder (one static SPMD program for all 8 cores).
# ----------------------------------------------------------------------------
def _build(meta, HID, OC):
    NP, shard, NBT, C = meta["NP"], meta["shard"], meta["NBT"], meta["C"]
    T, SCOLS = meta["T"], meta["SCOLS"]
    tiles = meta["tiles"]
    bucket_tiles = meta["bucket_tiles"]
    scol_off = meta["scol_off"]
    cb, chunk_nb = meta["cb"], meta["chunk_nb"]
    K = meta["K"]
    gorder, gpos, chunk_t0 = meta["gorder"], meta["gpos"], meta["chunk_t0"]

    nc = bacc.Bacc("TRN2", target_bir_lowering=False, debug=False,
                   num_devices=N_CORES, num_swdge_queues=4)

    msgs1_d = nc.dram_tensor("msgs1", [128, T * C], BF16, kind="ExternalInput")
    idx2_d = nc.dram_tensor("idx2", [128, T * 8], I16, kind="ExternalInput")
    s1_d = nc.dram_tensor("S1", [128, SCOLS], BF16, kind="ExternalInput")
    xT_d = nc.dram_tensor("xT_sh", [C, shard], BF16, kind="ExternalInput")
    ident_d = nc.dram_tensor("ident", [128, 128], BF16, kind="ExternalInput")
    w1l_d = nc.dram_tensor("W1_l", [C, HID], BF16, kind="ExternalInput")
    w1r_d = nc.dram_tensor("W1_r", [C, HID], BF16, kind="ExternalInput")
    w2l_d = nc.dram_tensor("W2_l", [HID, OC], BF16, kind="ExternalInput")
    w2r_d = nc.dram_tensor("W2_r", [HID, OC], BF16, kind="ExternalInput")
    b1_d = nc.dram_tensor("b1", [HID, 1], F32, kind="ExternalInput")
    b2_d = nc.dram_tensor("b2", [OC, 1], F32, kind="ExternalInput")
    out_d = nc.dram_tensor("out", [OC, shard], F32, kind="ExternalOutput")

    NC1 = _cdiv(T, CT1)
    # S1 split: batch-aligned column parts (each batch spans K*BATCH scols)
    bpp = _cdiv(NBT, SPARTS)                 # batches per S1 part
    cpp = bpp * K * BATCH                    # scols per part

    # batch -> last stream-chunk index needed (for just-in-time emission)
    last_tile_of_batch = [bucket_tiles[(b, K - 1)][-1] for b in range(NBT)]
    need_chunk = [t // CT1 for t in last_tile_of_batch]

    # Software-pipelined emission schedule.  Gather calls are paced at
    # CALL_RATE per batch iteration starting right after their chunk's AG;
    # a bucket's layer-2 matmuls are emitted GAP iterations after its last
    # gather call so the DMA has landed and the PE never stalls long.
    CALL_RATE = 0
    GAP = 4
    calls = []              # (chunk, s0, nt) in gather order
    for c in range(K):
        g0, g1 = chunk_t0[c], chunk_t0[c + 1]
        for s0 in range(g0, g1, CT):
            calls.append((c, s0, min(CT, g1 - s0)))
    call_iter = {}          # gather pos -> iteration its call was emitted
    gcall_at = {b: [] for b in range(NBT)}
    ci = 0
    for b in range(NBT):
        budget = CALL_RATE
        while ci < len(calls) and budget > 0:
            c, s0, nt = calls[ci]
            if b < cb[c + 1]:       # chunk's AG not yet emitted
                break
            gcall_at[b].append(calls[ci])
            for g in range(s0, s0 + nt):
                call_iter[g] = b
            ci += 1
            budget -= 1
    gcall_post = calls[ci:]
    for c, s0, nt in gcall_post:
        for g in range(s0, s0 + nt):
            call_iter[g] = NBT + 1000
    # bucket -> iteration when its last tile's call was emitted
    mm_at = {b: [] for b in range(NBT)}
    mm_post = []
    for c in range(K):
        for bb in range(NBT):
            last = max(call_iter[gpos[t]] for t in bucket_tiles[(bb, c)])
            if last + GAP < NBT:
                mm_at[last + GAP].append((c, bb))
            else:
                mm_post.append((c, bb))
    mm_post.sort(key=lambda cb_: (cb_[0], cb_[1]))

    with tile.TileContext(nc) as tc:
        with (
            tc.tile_pool(name="res", bufs=1) as rp,
            tc.tile_pool(name="m1p", bufs=3) as m1p,
            tc.tile_pool(name="m2p", bufs=12) as m2p,
            tc.tile_pool(name="stage", bufs=3) as stp,
            tc.tile_pool(name="pA", bufs=2, space="PSUM") as pA,
            tc.tile_pool(name="pB", bufs=2, space="PSUM") as pB,
            tc.tile_pool(name="pD", bufs=3, space="PSUM") as pD,
            tc.tile_pool(name="dram", bufs=1, space="DRAM") as dram_p,
        ):
            def load(shape, dtype, dram_t, name):
                # One-shot loads of persistent tiles on the ACT HWDGE ring:
                # keeps them off the sync ring which carries the msgs1 stream
                # and staging writes.
                t = rp.tile(shape, dtype, name=name, tag=name)
                nc.scalar.dma_start(t[:], dram_t.ap())
                return t

            # S1 parts loaded first (first L1 matmul waits only part 0)
            s1_parts = []
            for i in range(_cdiv(SCOLS, cpp)):
                c0 = i * cpp
                w = min(cpp, SCOLS - c0)
                t = rp.tile([128, w], BF16, name=f"s1p{i}", tag=f"s1p{i}")
                nc.scalar.dma_start(t[:], s1_d.ap()[:, c0:c0 + w])
                s1_parts.append(t)

            def s1_sl(t):
                # (part tile, local col offset) for tile id t
                off = scol_off[t]
                return s1_parts[off // cpp], off % cpp

            xT_sb = load([C, shard], BF16, xT_d, "xT_sb")
            w1l_sb = load([C, HID], BF16, w1l_d, "w1l_sb")
            w1r_sb = load([C, HID], BF16, w1r_d, "w1r_sb")
            w2l_sb = load([HID, OC], BF16, w2l_d, "w2l_sb")
            w2r_sb = load([HID, OC], BF16, w2r_d, "w2r_sb")
            b1_sb = load([HID, 1], F32, b1_d, "b1_sb")
            b2_sb = load([OC, 1], F32, b2_d, "b2_sb")
            ident_sb = load([128, 128], BF16, ident_d, "ident_sb")
            idx2_sb = load([128, T * 8], I16, idx2_d, "idx2_sb")

            hT_sb = rp.tile([HID, shard], BF16, name="hT_sb", tag="hT_sb")
            acc_sb = rp.tile([HID, shard], F32, name="acc_sb", tag="acc_sb")
            accb_sb = rp.tile([HID, shard], BF16, name="accb_sb",
                              tag="accb_sb")

            ag_in = [dram_p.tile([chunk_nb[c] * BATCH, 128], BF16,
                                 name=f"ag_in{c}") for c in range(K)]
            h_ch = [dram_p.tile([N_CORES * chunk_nb[c] * BATCH, 128], BF16,
                                name=f"h_ch{c}", addr_space="Shared")
                    for c in range(K)]

            chunks1 = {}
            next_c1 = [0]

            def emit_stream(upto):
                while next_c1[0] <= min(upto, NC1 - 1):
                    ci = next_c1[0]
                    c0 = ci * CT1
                    nt = min(CT1, T - c0)
                    m1 = m1p.tile([128, CT1 * C], BF16, name="m1", tag="m1")
                    nc.sync.dma_start(m1[:, :nt * C],
                                      msgs1_d.ap()[:, c0 * C:(c0 + nt) * C])
                    chunks1[ci] = m1
                    next_c1[0] += 1

            gtile = {}   # gather position -> (m2 buf, slice index)
            qn = [0]

            def emit_call(c, s0, nt):
                m2 = m2p.tile([128, CT, C], BF16, name="m2", tag="m2")
                nc.gpsimd.dma_gather(
                    out_ap=m2[:, :nt, :],
                    in_ap=h_ch[c][:],
                    idxs_ap=idx2_sb[:, s0 * 8:(s0 + nt) * 8],
                    num_idxs=nt * 128,
                    num_idxs_reg=nt * 128,
                    elem_size=C,
                    queue_num=qn[0] % 4,
                )
                qn[0] += 1
                for j in range(nt):
                    gtile[s0 + j] = (m2, j)

            def emit_mm(c, b):
                # layer-2 aggregation for bucket (c, b): matmuls + acc update
                bts = bucket_tiles[(b, c)]
                pb = pB.tile([128, BATCH], F32, name="pb", tag="pb")
                for ti, t in enumerate(bts):
                    _, _, a, w = tiles[t]
                    m2, j = gtile[gpos[t]]
                    sp, so = s1_sl(t)
                    nc.tensor.matmul(
                        pb[:, a:a + w], m2[:, j, :], sp[:, so:so + w],
                        start=True, stop=True)
                blk = slice(b * BATCH, (b + 1) * BATCH)
                if K == 1:
                    nc.scalar.activation(
                        accb_sb[:, blk], pb[:],
                        mybir.ActivationFunctionType.Copy)
                elif c == 0:
                    nc.scalar.activation(
                        acc_sb[:, blk], pb[:],
                        mybir.ActivationFunctionType.Copy)
                elif c < K - 1:
                    nc.vector.tensor_tensor(
                        acc_sb[:, blk], acc_sb[:, blk], pb[:],
                        mybir.AluOpType.add)
                else:
                    nc.vector.tensor_tensor(
                        accb_sb[:, blk], acc_sb[:, blk], pb[:],
                        mybir.AluOpType.add)

            # ---- layer 1 + staged AllGathers + overlapped layer-2 ----
            for b in range(NBT):
                emit_stream(need_chunk[b] + (2 if b + 1 < NBT else 0))

                blk = slice(b * BATCH, (b + 1) * BATCH)
                dp = pD.tile([128, BATCH], F32, name="dp", tag="dp")
                for c in range(K):
                    psum = pA.tile([128, BATCH], F32, name="psum1", tag="pa")
                    for t in bucket_tiles[(b, c)]:
                        _, _, a, w = tiles[t]
                        mt = chunks1[t // CT1][
                            :, (t % CT1) * C:(t % CT1 + 1) * C]
                        sp, so = s1_sl(t)
                        nc.tensor.matmul(
                            psum[:, a:a + w], mt, sp[:, so:so + w],
                            start=True, stop=True)
                    aggc = stp.tile([128, BATCH], BF16, name="aggc",
                                    tag="aggc")
                    nc.scalar.activation(
                        aggc[:], psum[:], mybir.ActivationFunctionType.Copy)
                    nc.tensor.matmul(dp[:HID, :], w1l_sb[:], aggc[:],
                                     start=(c == 0), stop=False)
                nc.tensor.matmul(dp[:HID, :], w1r_sb[:], xT_sb[:, blk],
                                 start=False, stop=True)
                nc.scalar.activation(
                    hT_sb[:, blk], dp[:HID, :],
                    mybir.ActivationFunctionType.Relu, bias=b1_sb[:])

                # stage h rows (node-major) for the AllGather
                cur = int(np.searchsorted(np.asarray(cb[1:]), b,
                                          side="right"))
                tp = pD.tile([128, 128], BF16, name="tp", tag="dp")
                nc.tensor.transpose(tp[:], hT_sb[:, blk], ident_sb[:])
                zs = stp.tile([128, 128], BF16, name="zs", tag="zs")
                nc.vector.tensor_copy(zs[:], tp[:])
                r0 = (b - cb[cur]) * BATCH
                nc.sync.dma_start(ag_in[cur][r0:r0 + BATCH, :], zs[:])

                # all chunks but the last AllGather as soon as staged; the
                # last chunk's AG is emitted post-loop so its dispatch-wait
                # (staging of the final batches) gates the gather calls
                # behind it on the Pool queue until layer 1 is fully done --
                # the gather DMA pattern thrashes HBM if run concurrently
                # with the msgs stream.
                if b == cb[cur + 1] - 1 and cur < K - 1:
                    nc.gpsimd.collective_compute(
                        "AllGather", mybir.AluOpType.bypass,
                        replica_groups=[list(range(N_CORES))],
                        ins=[ag_in[cur].opt()], outs=[h_ch[cur].opt()])

                for (cc, s0, nt) in gcall_at[b]:
                    emit_call(cc, s0, nt)
                for (cc, bb) in mm_at[b]:
                    emit_mm(cc, bb)

            nc.gpsimd.collective_compute(
                "AllGather", mybir.AluOpType.bypass,
                replica_groups=[list(range(N_CORES))],
                ins=[ag_in[K - 1].opt()], outs=[h_ch[K - 1].opt()])
            for (cc, s0, nt) in gcall_post:
                emit_call(cc, s0, nt)
            for (cc, bb) in mm_post:
                emit_mm(cc, bb)

            # ---- layer 2 dense (root term + bias + agg term) ----
            for b in range(NBT):
                blk = slice(b * BATCH, (b + 1) * BATCH)
                rp_ = pD.tile([128, BATCH], F32, name="rp", tag="dp")
                nc.tensor.matmul(rp_[:OC, :], w2l_sb[:], accb_sb[:, blk],
                                 start=True, stop=False)
                nc.tensor.matmul(rp_[:OC, :], w2r_sb[:], hT_sb[:, blk],
                                 start=False, stop=True)
                o1 = stp.tile([OC, BATCH], F32, name="o1", tag="o1")
                nc.scalar.activation(
                    o1[:], rp_[:OC, :],
                    mybir.ActivationFunctionType.Identity, bias=b2_sb[:])
                nc.sync.dma_start(out_d.ap()[:, blk], o1[:])

    nc.compile()
    return nc


_CACHE = {}


def _prepare(x, edge_index, W1_l, b1, W1_r, W2_l, b2, W2_r):
    x = np.asarray(x, np.float32)
    W1_l = np.asarray(W1_l, np.float32)
    W1_r = np.asarray(W1_r, np.float32)
    W2_l = np.asarray(W2_l, np.float32)
    W2_r = np.asarray(W2_r, np.float32)
    b1 = np.asarray(b1, np.float32)
    b2 = np.asarray(b2, np.float32)
    HID = W1_l.shape[1]
    OC = W2_l.shape[1]
    N = x.shape[0]

    per_core, perm, meta = _preprocess(x, edge_index)

    key = (meta["NP"], meta["T"], meta["SCOLS"],
           tuple(meta["tiles"]), HID, OC)
    if key not in _CACHE:
        _CACHE[key] = _build(meta, HID, OC)
    nc = _CACHE[key]

    ident = np.eye(128, dtype=np.float32).astype(NP_BF16)
    shared = {
        "ident": ident,
        "W1_l": W1_l.astype(NP_BF16), "W1_r": W1_r.astype(NP_BF16),
        "W2_l": W2_l.astype(NP_BF16), "W2_r": W2_r.astype(NP_BF16),
        "b1": b1.reshape(HID, 1).copy(), "b2": b2.reshape(OC, 1).copy(),
    }
    in_maps = []
    for k in range(N_CORES):
        m = dict(shared)
        m.update(per_core[k])
        in_maps.append(m)
    return nc, in_maps, perm, N


def kernel(x, edge_index, W1_l, b1, W1_r, W2_l, b2, W2_r):
    nc, in_maps, perm, N = _prepare(x, edge_index, W1_l, b1, W1_r,
                                    W2_l, b2, W2_r)
    res = run_bass_kernel_spmd(nc, in_maps, core_ids=list(range(N_CORES)))
    out_full = np.concatenate(
        [res.results[k]["out"] for k in range(N_CORES)], axis=1)
    return np.ascontiguousarray(out_full.T[perm[:N]].astype(np.float32))


# revision 14
# speedup vs baseline: 1.0464x; 1.0464x over previous
# Trainium2 Bass kernel for a 2-layer GraphSAGE encoder (SAGEConv mean aggr).
#
#   h   = relu(mean_nbr(x) @ W1_l + b1 + x @ W1_r)
#   out = mean_nbr(h) @ W2_l + b2 + h @ W2_r
#
# Sharding: data-parallel over destination nodes (8 cores); host permutes node
# ids (degree-balanced snake deal), pads N to 8*shard, core k owns dst rows
# [k*shard,(k+1)*shard).
#
# Layer 1 messages x[src] are host-packed into a contiguous bf16 slot stream
# and streamed sequentially (no runtime gather); one PE matmul per 128-slot
# tile against segment matrix S1 (holding 1/deg) accumulates agg per dst.
#
# Layer 2 messages (h[src]) are gathered at runtime. To overlap the gather
# with layer 1, source nodes are split into K chunks by local index range;
# tiles are chunk-pure, and h is AllGathered chunk-by-chunk as soon as each
# chunk's dense layer completes, so chunk-c gathers run while the layer-1
# stream for later chunks is still going.  Gathered rows are full h rows
# (256B singles; per-chunk row index fits int16), aggregated per (chunk,
# batch) bucket into PSUM and accumulated into an SBUF f32 accumulator.
# The layer-2 matmuls for chunk c are emitted one chunk-span later so the PE
# stream never stalls on gather data and m2 pool buffers recycle in time.
# Small linear weights are replicated; outputs written [OC, shard]
# column-major and transposed on the host.
import os
import sys
import numpy as np

for _p in ("/opt/trn_rl_repo",):
    if _p not in sys.path and os.path.isdir(_p):
        sys.path.append(_p)

import concourse.bass as bass
import concourse.bacc as bacc
import concourse.mybir as mybir
from concourse import tile
from concourse.bass_utils import run_bass_kernel_spmd

F32 = mybir.dt.float32
BF16 = mybir.dt.bfloat16
I16 = mybir.dt.int16
NP_BF16 = mybir.dt.np(BF16)

N_CORES = 8
BATCH = 128      # dst nodes per aggregation batch (PSUM tile width)
K = 2            # source chunks (AllGather pipeline depth)
CT = 8           # L2 gather call size in 128-slot tiles
CT1 = 32         # L1 stream chunk size in tiles (1MB DMAs)
SPARTS = 8       # S1 load splits (batch-aligned)


def _cdiv(a, b):
    return -(-a // b)


# ----------------------------------------------------------------------------
# Host-side graph preprocessing (index manipulation / data staging only).
# ----------------------------------------------------------------------------
def _preprocess(x, edge_index):
    x = np.asarray(x, np.float32)
    ei = np.asarray(edge_index, np.int64)
    N, C = x.shape
    E = ei.shape[1]
    src, dst = ei[0], ei[1]

    shard = _cdiv(_cdiv(N, N_CORES), BATCH) * BATCH
    NP = shard * N_CORES
    NBT = shard // BATCH

    # chunk boundaries in batch units (Ke chunks over NBT batches)
    Ke = max(1, min(K, NBT))
    cb = [round(c * NBT / Ke) for c in range(Ke + 1)]
    chunk_nb = [cb[c + 1] - cb[c] for c in range(Ke)]

    deg = np.bincount(dst, minlength=N).astype(np.int64)
    recip_full = (1.0 / np.maximum(deg, 1)).astype(np.float32)

    # Degree-balanced snake deal over (core, batch-of-128) bins.
    nbins = N_CORES * NBT
    order = np.argsort(-deg, kind="stable")
    i = np.arange(N)
    r = i // nbins
    p = i % nbins
    binidx = np.where(r % 2 == 0, p, nbins - 1 - p)
    core_b = binidx % N_CORES
    bat_b = binidx // N_CORES
    newid = core_b * shard + bat_b * BATCH + r
    perm = np.empty(N, np.int64)
    perm[order] = newid

    psrc = perm[src]
    pdst = perm[dst]

    x_tab = np.zeros((NP, C), np.float32)
    x_tab[perm] = x
    x_tab16 = x_tab.astype(NP_BF16)
    recip_bc = np.zeros(NP, np.float32)
    recip_bc[perm] = recip_full

    # source chunk of each edge (by src's local batch index on its owner core)
    src_local = psrc % shard
    src_bat = src_local // BATCH
    cb_arr = np.asarray(cb[1:])                        # upper bounds
    src_chunk = np.searchsorted(cb_arr, src_bat, side="right")
    # row of src within h_chunk_c: owner_core * chunk_nodes + (local - start)
    src_core = psrc // shard
    src_row = np.empty(E, np.int64)
    for c in range(Ke):
        m = src_chunk == c
        src_row[m] = (src_core[m] * (chunk_nb[c] * BATCH)
                      + src_local[m] - cb[c] * BATCH)
        assert src_row[m].max(initial=0) < 32768, "chunk row exceeds int16"

    core_of = pdst // shard
    local = pdst % shard

    # per-(core, local-dst, chunk) degree
    keyd = (core_of * shard + local) * Ke + src_chunk
    degs = np.bincount(keyd, minlength=N_CORES * shard * Ke)
    degs = degs.reshape(N_CORES, shard, Ke)              # [core, local, chunk]
    assert degs.sum(axis=2).max() <= 128, "single dst degree exceeds one tile"

    # Structural tile plan, uniform across cores. Stream (batch-major) order:
    # for each batch b, for each chunk c, greedily pack the 128 dst columns
    # (zero-width allowed; every dst gets a column in every bucket) into tiles
    # where every core's slot total fits 128.
    tiles = []            # (b, c, a, w) in stream-emission order
    bucket_tiles = {}     # (b, c) -> list of tile ids
    for b in range(NBT):
        for c in range(Ke):
            d = degs[:, b * BATCH:(b + 1) * BATCH, c]   # [core, 128]
            csum = np.concatenate(
                [np.zeros((N_CORES, 1), np.int64), np.cumsum(d, axis=1)],
                axis=1)
            tl = []
            a = 0
            while a < BATCH:
                base = csum[:, a]
                w = 1
                while a + w < BATCH and \
                        ((csum[:, a + w + 1] - base) <= 128).all():
                    w += 1
                tl.append((b, c, a, w))
                a += w
            bucket_tiles[(b, c)] = list(range(len(tiles), len(tiles) + len(tl)))
            tiles.extend(tl)
    T = len(tiles)

    scol_off = np.zeros(T + 1, np.int64)
    for t, (b, c, a, w) in enumerate(tiles):
        scol_off[t + 1] = scol_off[t] + w
    SCOLS = int(scol_off[-1])
    assert SCOLS == NBT * Ke * BATCH

    # gather (chunk-major) tile order
    gorder = []           # gather position -> tile id
    for c in range(Ke):
        for b in range(NBT):
            gorder.extend(bucket_tiles[(b, c)])
    gpos = np.empty(T, np.int64)
    gpos[np.asarray(gorder)] = np.arange(T)
    chunk_t0 = [0] * (Ke + 1)    # gather-pos range per chunk
    for c in range(Ke):
        nt_c = sum(len(bucket_tiles[(b, c)]) for b in range(NBT))
        chunk_t0[c + 1] = chunk_t0[c] + nt_c

    # --- per-core slot/S content -------------------------------------------
    # edges sorted by (core, local dst, chunk) with rank within each group
    keye = (core_of * shard + local) * Ke + src_chunk
    ordr = np.argsort(keye, kind="stable")
    psrc_s = psrc[ordr]
    srcrow_s = src_row[ordr]
    keye_s = keye[ordr]
    starts = np.concatenate([[0], np.cumsum(degs.reshape(-1))])
    rank = np.arange(E) - starts[keye_s]
    core_e = keye_s // (shard * Ke)
    loc_e = (keye_s // Ke) % shard
    chk_e = keye_s % Ke

    def wrap_idx(a_):
        return np.ascontiguousarray(
            np.tile(a_.reshape(-1, 16).T, (8, 1)).astype(np.int16))

    per_core = []
    for k in range(N_CORES):
        slot_base = np.zeros((shard, Ke), np.int64)
        S1 = np.zeros((128, SCOLS), np.float32)
        for t, (b, c, a, w) in enumerate(tiles):
            dloc = b * BATCH + a
            dsl = degs[k, dloc:dloc + w, c]
            offs = np.concatenate([[0], np.cumsum(dsl)])
            assert offs[-1] <= 128
            slot_base[dloc:dloc + w, c] = t * 128 + offs[:-1]
            nz = np.nonzero(dsl)[0]
            for j in nz:
                S1[offs[j]:offs[j + 1], scol_off[t] + j] = \
                    recip_bc[k * shard + dloc + j]
        m = core_e == k
        slot = slot_base[loc_e[m], chk_e[m]] + rank[m]
        slotsrc = np.zeros(T * 128, np.int64)
        slotsrc[slot] = psrc_s[m]
        slotrow = np.zeros(T * 128, np.int64)
        slotrow[slot] = srcrow_s[m]

        src_grid = slotsrc.reshape(T, 128).T            # [128, T] stream order
        msgs1 = x_tab16[src_grid].reshape(128, T * C)   # [128, T*C] bf16

        # gather-order idx: per gather position g, the tile gorder[g]
        rows_g = slotrow.reshape(T, 128)[np.asarray(gorder)]  # [T,128] g-order
        idx2 = wrap_idx(rows_g.reshape(-1))             # [128, T*8] int16

        ent = {
            "msgs1": np.ascontiguousarray(msgs1),
            "idx2": idx2,
            "S1": S1.astype(NP_BF16),
            "xT_sh": np.ascontiguousarray(
                x_tab16.T[:, k * shard:(k + 1) * shard]),
        }
        per_core.append(ent)

    meta = dict(NP=NP, shard=shard, NBT=NBT, C=C, T=T, SCOLS=SCOLS, K=Ke,
                tiles=tiles, bucket_tiles=bucket_tiles,
                scol_off=scol_off.tolist(), cb=cb, chunk_nb=chunk_nb,
                gorder=gorder, gpos=gpos.tolist(), chunk_t0=chunk_t0)
    return per_core, perm, meta


# ----------------------------------------------------------------------------
# Bass program builder (one static SPMD program for all 8 cores).
# ----------------------------------------------------------------------------
def _build(meta, HID, OC):
    NP, shard, NBT, C = meta["NP"], meta["shard"], meta["NBT"], meta["C"]
    T, SCOLS = meta["T"], meta["SCOLS"]
    tiles = meta["tiles"]
    bucket_tiles = meta["bucket_tiles"]
    scol_off = meta["scol_off"]
    cb, chunk_nb = meta["cb"], meta["chunk_nb"]
    K = meta["K"]
    gorder, gpos, chunk_t0 = meta["gorder"], meta["gpos"], meta["chunk_t0"]

    nc = bacc.Bacc("TRN2", target_bir_lowering=False, debug=False,
                   num_devices=N_CORES, num_swdge_queues=4)

    msgs1_d = nc.dram_tensor("msgs1", [128, T * C], BF16, kind="ExternalInput")
    idx2_d = nc.dram_tensor("idx2", [128, T * 8], I16, kind="ExternalInput")
    s1_d = nc.dram_tensor("S1", [128, SCOLS], BF16, kind="ExternalInput")
    xT_d = nc.dram_tensor("xT_sh", [C, shard], BF16, kind="ExternalInput")
    ident_d = nc.dram_tensor("ident", [128, 128], BF16, kind="ExternalInput")
    w1l_d = nc.dram_tensor("W1_l", [C, HID], BF16, kind="ExternalInput")
    w1r_d = nc.dram_tensor("W1_r", [C, HID], BF16, kind="ExternalInput")
    w2l_d = nc.dram_tensor("W2_l", [HID, OC], BF16, kind="ExternalInput")
    w2r_d = nc.dram_tensor("W2_r", [HID, OC], BF16, kind="ExternalInput")
    b1_d = nc.dram_tensor("b1", [HID, 1], F32, kind="ExternalInput")
    b2_d = nc.dram_tensor("b2", [OC, 1], F32, kind="ExternalInput")
    out_d = nc.dram_tensor("out", [OC, shard], F32, kind="ExternalOutput")

    NC1 = _cdiv(T, CT1)
    # S1 split: batch-aligned column parts (each batch spans K*BATCH scols)
    bpp = _cdiv(NBT, SPARTS)                 # batches per S1 part
    cpp = bpp * K * BATCH                    # scols per part

    # batch -> last stream-chunk index needed (for just-in-time emission)
    last_tile_of_batch = [bucket_tiles[(b, K - 1)][-1] for b in range(NBT)]
    need_chunk = [t // CT1 for t in last_tile_of_batch]

    # Software-pipelined emission schedule.  Gather calls are paced at
    # CALL_RATE per batch iteration starting right after their chunk's AG;
    # a bucket's layer-2 matmuls are emitted GAP iterations after its last
    # gather call so the DMA has landed and the PE never stalls long.
    CALL_RATE = 0
    GAP = 4
    # Gather the LAST chunk first: its AllGather depends on the final
    # batches' staging, so the first gather call (and everything behind it
    # on the in-order Pool queue) cannot start until layer 1 is fully done.
    # Running the gathers concurrently with the msgs stream thrashes HBM.
    gchunks = [K - 1] + list(range(K - 1))
    grole = {c: i for i, c in enumerate(gchunks)}   # gather position of chunk
    calls = []              # (chunk, s0, nt) in gather order
    for c in gchunks:
        g0, g1 = chunk_t0[c], chunk_t0[c + 1]
        for s0 in range(g0, g1, CT):
            calls.append((c, s0, min(CT, g1 - s0)))
    call_iter = {}          # gather pos -> iteration its call was emitted
    gcall_at = {b: [] for b in range(NBT)}
    ci = 0
    for b in range(NBT):
        budget = CALL_RATE
        while ci < len(calls) and budget > 0:
            c, s0, nt = calls[ci]
            if b < cb[c + 1]:       # chunk's AG not yet emitted
                break
            gcall_at[b].append(calls[ci])
            for g in range(s0, s0 + nt):
                call_iter[g] = b
            ci += 1
            budget -= 1
    gcall_post = calls[ci:]
    for c, s0, nt in gcall_post:
        for g in range(s0, s0 + nt):
            call_iter[g] = NBT + 1000
    # bucket -> iteration when its last tile's call was emitted
    mm_at = {b: [] for b in range(NBT)}
    mm_post = []
    for c in range(K):
        for bb in range(NBT):
            last = max(call_iter[gpos[t]] for t in bucket_tiles[(bb, c)])
            if last + GAP < NBT:
                mm_at[last + GAP].append((c, bb))
            else:
                mm_post.append((c, bb))
    mm_post.sort(key=lambda cb_: (grole[cb_[0]], cb_[1]))

    with tile.TileContext(nc) as tc:
        with (
            tc.tile_pool(name="res", bufs=1) as rp,
            tc.tile_pool(name="m1p", bufs=3) as m1p,
            tc.tile_pool(name="m2p", bufs=12) as m2p,
            tc.tile_pool(name="stage", bufs=3) as stp,
            tc.tile_pool(name="pA", bufs=2, space="PSUM") as pA,
            tc.tile_pool(name="pB", bufs=2, space="PSUM") as pB,
            tc.tile_pool(name="pD", bufs=3, space="PSUM") as pD,
            tc.tile_pool(name="dram", bufs=1, space="DRAM") as dram_p,
        ):
            def load(shape, dtype, dram_t, name):
                # One-shot loads of persistent tiles on the ACT HWDGE ring:
                # keeps them off the sync ring which carries the msgs1 stream
                # and staging writes.
                t = rp.tile(shape, dtype, name=name, tag=name)
                nc.scalar.dma_start(t[:], dram_t.ap())
                return t

            # S1 parts loaded first (first L1 matmul waits only part 0)
            s1_parts = []
            for i in range(_cdiv(SCOLS, cpp)):
                c0 = i * cpp
                w = min(cpp, SCOLS - c0)
                t = rp.tile([128, w], BF16, name=f"s1p{i}", tag=f"s1p{i}")
                nc.scalar.dma_start(t[:], s1_d.ap()[:, c0:c0 + w])
                s1_parts.append(t)

            def s1_sl(t):
                # (part tile, local col offset) for tile id t
                off = scol_off[t]
                return s1_parts[off // cpp], off % cpp

            xT_sb = load([C, shard], BF16, xT_d, "xT_sb")
            w1l_sb = load([C, HID], BF16, w1l_d, "w1l_sb")
            w1r_sb = load([C, HID], BF16, w1r_d, "w1r_sb")
            w2l_sb = load([HID, OC], BF16, w2l_d, "w2l_sb")
            w2r_sb = load([HID, OC], BF16, w2r_d, "w2r_sb")
            b1_sb = load([HID, 1], F32, b1_d, "b1_sb")
            b2_sb = load([OC, 1], F32, b2_d, "b2_sb")
            ident_sb = load([128, 128], BF16, ident_d, "ident_sb")
            idx2_sb = load([128, T * 8], I16, idx2_d, "idx2_sb")

            hT_sb = rp.tile([HID, shard], BF16, name="hT_sb", tag="hT_sb")
            acc_sb = rp.tile([HID, shard], F32, name="acc_sb", tag="acc_sb")
            accb_sb = rp.tile([HID, shard], BF16, name="accb_sb",
                              tag="accb_sb")

            ag_in = [dram_p.tile([chunk_nb[c] * BATCH, 128], BF16,
                                 name=f"ag_in{c}") for c in range(K)]
            h_ch = [dram_p.tile([N_CORES * chunk_nb[c] * BATCH, 128], BF16,
                                name=f"h_ch{c}", addr_space="Shared")
                    for c in range(K)]

            chunks1 = {}
            next_c1 = [0]

            def emit_stream(upto):
                while next_c1[0] <= min(upto, NC1 - 1):
                    ci = next_c1[0]
                    c0 = ci * CT1
                    nt = min(CT1, T - c0)
                    m1 = m1p.tile([128, CT1 * C], BF16, name="m1", tag="m1")
                    nc.sync.dma_start(m1[:, :nt * C],
                                      msgs1_d.ap()[:, c0 * C:(c0 + nt) * C])
                    chunks1[ci] = m1
                    next_c1[0] += 1

            gtile = {}   # gather position -> (m2 buf, slice index)
            qn = [0]

            def emit_call(c, s0, nt):
                m2 = m2p.tile([128, CT, C], BF16, name="m2", tag="m2")
                nc.gpsimd.dma_gather(
                    out_ap=m2[:, :nt, :],
                    in_ap=h_ch[c][:],
                    idxs_ap=idx2_sb[:, s0 * 8:(s0 + nt) * 8],
                    num_idxs=nt * 128,
                    num_idxs_reg=nt * 128,
                    elem_size=C,
                    queue_num=qn[0] % 4,
                )
                qn[0] += 1
                for j in range(nt):
                    gtile[s0 + j] = (m2, j)

            def emit_mm(c, b):
                # layer-2 aggregation for bucket (c, b): matmuls + acc update
                bts = bucket_tiles[(b, c)]
                pb = pB.tile([128, BATCH], F32, name="pb", tag="pb")
                for ti, t in enumerate(bts):
                    _, _, a, w = tiles[t]
                    m2, j = gtile[gpos[t]]
                    sp, so = s1_sl(t)
                    nc.tensor.matmul(
                        pb[:, a:a + w], m2[:, j, :], sp[:, so:so + w],
                        start=True, stop=True)
                blk = slice(b * BATCH, (b + 1) * BATCH)
                r = grole[c]
                if K == 1:
                    nc.scalar.activation(
                        accb_sb[:, blk], pb[:],
                        mybir.ActivationFunctionType.Copy)
                elif r == 0:
                    nc.scalar.activation(
                        acc_sb[:, blk], pb[:],
                        mybir.ActivationFunctionType.Copy)
                elif r < K - 1:
                    nc.vector.tensor_tensor(
                        acc_sb[:, blk], acc_sb[:, blk], pb[:],
                        mybir.AluOpType.add)
                else:
                    nc.vector.tensor_tensor(
                        accb_sb[:, blk], acc_sb[:, blk], pb[:],
                        mybir.AluOpType.add)

            # ---- layer 1 + staged AllGathers + overlapped layer-2 ----
            for b in range(NBT):
                emit_stream(need_chunk[b] + (2 if b + 1 < NBT else 0))

                blk = slice(b * BATCH, (b + 1) * BATCH)
                dp = pD.tile([128, BATCH], F32, name="dp", tag="dp")
                for c in range(K):
                    psum = pA.tile([128, BATCH], F32, name="psum1", tag="pa")
                    for t in bucket_tiles[(b, c)]:
                        _, _, a, w = tiles[t]
                        mt = chunks1[t // CT1][
                            :, (t % CT1) * C:(t % CT1 + 1) * C]
                        sp, so = s1_sl(t)
                        nc.tensor.matmul(
                            psum[:, a:a + w], mt, sp[:, so:so + w],
                            start=True, stop=True)
                    aggc = stp.tile([128, BATCH], BF16, name="aggc",
                                    tag="aggc")
                    nc.scalar.activation(
                        aggc[:], psum[:], mybir.ActivationFunctionType.Copy)
                    nc.tensor.matmul(dp[:HID, :], w1l_sb[:], aggc[:],
                                     start=(c == 0), stop=False)
                nc.tensor.matmul(dp[:HID, :], w1r_sb[:], xT_sb[:, blk],
                                 start=False, stop=True)
                nc.scalar.activation(
                    hT_sb[:, blk], dp[:HID, :],
                    mybir.ActivationFunctionType.Relu, bias=b1_sb[:])

                # stage h rows (node-major) for the AllGather
                cur = int(np.searchsorted(np.asarray(cb[1:]), b,
                                          side="right"))
                tp = pD.tile([128, 128], BF16, name="tp", tag="dp")
                nc.tensor.transpose(tp[:], hT_sb[:, blk], ident_sb[:])
                zs = stp.tile([128, 128], BF16, name="zs", tag="zs")
                nc.vector.tensor_copy(zs[:], tp[:])
                r0 = (b - cb[cur]) * BATCH
                nc.sync.dma_start(ag_in[cur][r0:r0 + BATCH, :], zs[:])

                # all chunks but the last AllGather as soon as staged; the
                # last chunk's AG is emitted post-loop so its dispatch-wait
                # (staging of the final batches) gates the gather calls
                # behind it on the Pool queue until layer 1 is fully done --
                # the gather DMA pattern thrashes HBM if run concurrently
                # with the msgs stream.
                if b == cb[cur + 1] - 1 and cur < K - 1:
                    nc.gpsimd.collective_compute(
                        "AllGather", mybir.AluOpType.bypass,
                        replica_groups=[list(range(N_CORES))],
                        ins=[ag_in[cur].opt()], outs=[h_ch[cur].opt()])

                for (cc, s0, nt) in gcall_at[b]:
                    emit_call(cc, s0, nt)
                for (cc, bb) in mm_at[b]:
                    emit_mm(cc, bb)

            nc.gpsimd.collective_compute(
                "AllGather", mybir.AluOpType.bypass,
                replica_groups=[list(range(N_CORES))],
                ins=[ag_in[K - 1].opt()], outs=[h_ch[K - 1].opt()])
            for (cc, s0, nt) in gcall_post:
                emit_call(cc, s0, nt)
            for (cc, bb) in mm_post:
                emit_mm(cc, bb)

            # ---- layer 2 dense (root term + bias + agg term) ----
            for b in range(NBT):
                blk = slice(b * BATCH, (b + 1) * BATCH)
                rp_ = pD.tile([128, BATCH], F32, name="rp", tag="dp")
                nc.tensor.matmul(rp_[:OC, :], w2l_sb[:], accb_sb[:, blk],
                                 start=True, stop=False)
                nc.tensor.matmul(rp_[:OC, :], w2r_sb[:], hT_sb[:, blk],
                                 start=False, stop=True)
                o1 = stp.tile([OC, BATCH], F32, name="o1", tag="o1")
                nc.scalar.activation(
                    o1[:], rp_[:OC, :],
                    mybir.ActivationFunctionType.Identity, bias=b2_sb[:])
                nc.sync.dma_start(out_d.ap()[:, blk], o1[:])

    nc.compile()
    return nc


_CACHE = {}


def _prepare(x, edge_index, W1_l, b1, W1_r, W2_l, b2, W2_r):
    x = np.asarray(x, np.float32)
    W1_l = np.asarray(W1_l, np.float32)
    W1_r = np.asarray(W1_r, np.float32)
    W2_l = np.asarray(W2_l, np.float32)
    W2_r = np.asarray(W2_r, np.float32)
    b1 = np.asarray(b1, np.float32)
    b2 = np.asarray(b2, np.float32)
    HID = W1_l.shape[1]
    OC = W2_l.shape[1]
    N = x.shape[0]

    per_core, perm, meta = _preprocess(x, edge_index)

    key = (meta["NP"], meta["T"], meta["SCOLS"],
           tuple(meta["tiles"]), HID, OC)
    if key not in _CACHE:
        _CACHE[key] = _build(meta, HID, OC)
    nc = _CACHE[key]

    ident = np.eye(128, dtype=np.float32).astype(NP_BF16)
    shared = {
        "ident": ident,
        "W1_l": W1_l.astype(NP_BF16), "W1_r": W1_r.astype(NP_BF16),
        "W2_l": W2_l.astype(NP_BF16), "W2_r": W2_r.astype(NP_BF16),
        "b1": b1.reshape(HID, 1).copy(), "b2": b2.reshape(OC, 1).copy(),
    }
    in_maps = []
    for k in range(N_CORES):
        m = dict(shared)
        m.update(per_core[k])
        in_maps.append(m)
    return nc, in_maps, perm, N


def kernel(x, edge_index, W1_l, b1, W1_r, W2_l, b2, W2_r):
    nc, in_maps, perm, N = _prepare(x, edge_index, W1_l, b1, W1_r,
                                    W2_l, b2, W2_r)
    res = run_bass_kernel_spmd(nc, in_maps, core_ids=list(range(N_CORES)))
    out_full = np.concatenate(
        [res.results[k]["out"] for k in range(N_CORES)], axis=1)
    return np.ascontiguousarray(out_full.T[perm[:N]].astype(np.float32))
